# revision 30
# baseline (speedup 1.0000x reference)
"""Multi-head attention (16 heads, d_model=1024, T=2048, B=2) on 8 trn2 NeuronCores.

Sharding: core c -> batch c//4, head-group c%4 (4 heads of 64 dims each).
Each core computes q/k/v projections for its 4 heads on its batch, full
softmax attention for those heads, and a partial output projection
(row-parallel Wo).  Host sums the 4 partials per batch and adds the bias.

All matmuls run in bf16 (fp32 PSUM accumulation).  Scores are computed
transposed (ST[u,t] = sum_s k[u,s] q[t,s]) so that:
  - softmax sum over u is obtained from the attention*V matmul itself by
    appending a ones-column to V (row 64 of the av output = denominator),
  - no transposes of the 2048x2048 attention matrix are ever needed.
The 1/sqrt(d_model) scale is folded into Wq on the host.
"""

import math
import types
import sys

import numpy as np
import ml_dtypes

B = 2
T = 1024 * 2  # 2048 sequence
K = 1024  # model dim
H = 16  # heads
S = K // H  # 64 head dim
HPC = 4  # heads per core
NCORES = 8

_BF16 = ml_dtypes.bfloat16
import os as _os

_RECIP_FAST = _os.environ.get("RECIP_FAST", "0") == "1"


def _install_drain_split_patch():
    """walrus in this container rejects >1 sync-wait on the final tile drain;
    split the waits one-per-drain-instruction (all before the end barrier)."""
    import concourse.tile as tile
    import concourse.mybir as mybir
    from concourse.vector_clock import ScopedClock

    if getattr(tile.TileContext, "_drain_split_patched", False):
        return

    def _patched_dab(self, tick_clock, wait_clock):
        drain_inst = self.nc.sync.drain()
        wait_clock.add_sem_waits(
            drain_inst.ins, ScopedClock({None: tick_clock.global_clock})
        )
        si = drain_inst.ins.sync_info
        waits = list(si.on_wait) if si is not None else []
        if len(waits) > 1:
            si.on_wait = waits[:1]
            for w in waits[1:]:
                extra = self.nc.sync.drain()
                esi = extra.ins.sync_info
                if esi is None:
                    extra.ins.sync_info = mybir.SyncInfo(on_update=[], on_wait=[w])
                else:
                    esi.on_wait = [w]
        self.nc.all_engine_barrier()
        assert self.sems is not None
        popped = self.nc._tile_sem_poison_stack.pop()
        assert popped is self._sem_poison
        self.nc.clear_and_free_semaphores(list(self.sems.allocated().values()))
        self.nc.all_engine_barrier()

    tile.TileContext._drain_and_barrier = _patched_dab
    tile.TileContext._drain_split_patched = True


def build_program():
    """Build the single-core Bass program (same program on all 8 cores)."""
    import concourse.bass as bass
    import concourse.mybir as mybir
    import concourse.tile as tile
    from concourse import bacc

    dt = mybir.dt
    AF = mybir.ActivationFunctionType
    Alu = mybir.AluOpType

    nc = bacc.Bacc()

    xT = nc.dram_tensor("xT", [K, T], dt.bfloat16, kind="ExternalInput")
    wq = nc.dram_tensor("wq", [K, 256], dt.bfloat16, kind="ExternalInput")
    wk = nc.dram_tensor("wk", [K, 256], dt.bfloat16, kind="ExternalInput")
    wv = nc.dram_tensor("wv", [K, 256], dt.bfloat16, kind="ExternalInput")
    wo = nc.dram_tensor("wo", [256, K], dt.bfloat16, kind="ExternalInput")
    out = nc.dram_tensor("out", [T, K], dt.float32, kind="ExternalOutput")

    KT = K // 128  # 8 k tiles
    TB = T // 128  # 16 t blocks
    VW = 65  # v columns per head incl ones col
    VROW = HPC * VW  # 260 per u-block row

    with tile.TileContext(nc) as tc:
        with (
            tc.tile_pool(name="xt", bufs=KT) as xt_pool,
            tc.tile_pool(name="w", bufs=3) as w_pool,
            tc.tile_pool(name="wo", bufs=2) as wo_pool,
            tc.tile_pool(name="qk", bufs=4) as qk_pool,
            tc.tile_pool(name="v", bufs=1) as v_pool,
            tc.tile_pool(name="yt", bufs=2) as yt_pool,
            tc.tile_pool(name="e", bufs=4) as e_pool,
            tc.tile_pool(name="dinv", bufs=4) as dinv_pool,
            tc.tile_pool(name="avs", bufs=4) as avs_pool,
            tc.tile_pool(name="osb", bufs=3) as osb_pool,
            tc.tile_pool(name="ps1", bufs=4, space="PSUM") as ps1_pool,
            tc.tile_pool(name="pst", bufs=2, space="PSUM") as pst_pool,
        ):
            # ---- loads ----
            xt = []
            for a in range(KT):
                t = xt_pool.tile([128, T], dt.bfloat16, tag="xt")
                nc.sync.dma_start(t[:], xT[a * 128 : (a + 1) * 128, :])
                xt.append(t)

            w_sb = {}
            for name, dram in (("q", wq), ("k", wk), ("v", wv)):
                t = w_pool.tile([128, KT * 256], dt.bfloat16, tag="w")
                nc.sync.dma_start(
                    t[:].rearrange("p (a c) -> p a c", a=KT),
                    dram.rearrange("(a p) c -> p a c", p=128),
                )
                w_sb[name] = t

            wo_sb = []
            for i in range(2):
                t = wo_pool.tile([128, K], dt.bfloat16, tag="wo")
                nc.sync.dma_start(t[:], wo[i * 128 : (i + 1) * 128, :])
                wo_sb.append(t)

            # v with ones columns: [128, 16 u-blocks * (4 heads * 65)]
            v_sb = v_pool.tile([128, TB * VROW], dt.bfloat16, tag="v")
            ones_ap = v_sb[:].rearrange(
                "p (u h c) -> p u h c", u=TB, h=HPC
            )[:, :, :, S : S + 1]
            nc.vector.memset(ones_ap, 1.0)

            # selector matrices for PE-based partition broadcast of 1/D:
            # dinv rows live at partitions j = hl*2 + c; SEL_c.T @ dinv puts
            # row (hl*2+c) broadcast over out partitions hl*64..hl*64+63.
            sel = []
            for c in range(2):
                s = v_pool.tile([128, 128], dt.bfloat16, tag=f"sel{c}", name=f"sel_{c}")
                nc.vector.memset(s[:], 0.0)
                nc.vector.memset(s[32 * c : 32 * c + 1, 0:64], 1.0)
                nc.vector.memset(s[64 + 32 * c : 64 + 32 * c + 1, 64:128], 1.0)
                sel.append(s)

            # ---- projections ----
            # v first (attention needs it for every head); q/k per pair on
            # demand so attention of pair 0 can start as early as possible.
            for tb in range(TB):
                ps = ps1_pool.tile([128, 256], dt.float32, tag="ps1")
                for a in range(KT):
                    nc.tensor.matmul(
                        ps[:],
                        xt[a][:, tb * 128 : (tb + 1) * 128],
                        w_sb["v"][:, a * 256 : (a + 1) * 256],
                        start=(a == 0),
                        stop=(a == KT - 1),
                    )
                dst = v_sb[:].rearrange("p (u h c) -> p u h c", u=TB, h=HPC)[
                    :, tb, :, 0:S
                ]
                nc.vector.tensor_copy(dst, ps[:].rearrange("p (h c) -> p h c", h=HPC))

            qt_sb = [None, None]  # per head pair: [128, T], rows 2x64 head dims
            kt_sb = [None, None]

            def qk_proj(hp):
                qt = qk_pool.tile([128, T], dt.bfloat16, tag="qk", name=f"qt_{hp}")
                kt = qk_pool.tile([128, T], dt.bfloat16, tag="qk", name=f"kt_{hp}")
                qt_sb[hp] = qt
                kt_sb[hp] = kt
                for which, dst in (("q", qt), ("k", kt)):
                    # weights stay stationary across the 4 output chunks
                    pss = [
                        ps1_pool.tile(
                            [128, 512], dt.float32, tag="ps1", name=f"pp_{hp}_{which}_{c}"
                        )
                        for c in range(4)
                    ]
                    for a in range(KT):
                        for c in range(4):
                            nc.tensor.matmul(
                                pss[c][:],
                                w_sb[which][:, a * 256 + hp * 128 : a * 256 + hp * 128 + 128],
                                xt[a][:, c * 512 : (c + 1) * 512],
                                start=(a == 0),
                                stop=(a == KT - 1),
                            )
                    for c in range(4):
                        tsl = slice(c * 512, (c + 1) * 512)
                        if which == "q":
                            nc.scalar.copy(dst[:, tsl], pss[c][:])
                        else:
                            nc.vector.tensor_copy(dst[:, tsl], pss[c][:])

            # ---- attention (t-half outer; heads of a pair interleaved) ----
            # yt[hp]: [128, T] bf16, rows (h%2)*64+s hold y^T for the pair
            yt_sb = [
                yt_pool.tile([128, T], dt.bfloat16, tag="yt", name=f"yt_{hp}")
                for hp in range(2)
            ]

            def attention_uloop(hp, th):
                t0 = th * 1024
                av = {}
                for hl in range(2):
                    for c in range(2):
                        av[(hl, c)] = ps1_pool.tile(
                            [65, 512], dt.float32, tag="ps1", name=f"av_{hp}_{th}_{hl}_{c}"
                        )
                for ub in range(TB):
                    es = []
                    for hl in range(2):
                        st = pst_pool.tile(
                            [128, 1024], dt.float32, tag="st", name=f"st_{hp}_{th}_{ub}_{hl}"
                        )
                        for c in range(2):
                            nc.tensor.matmul(
                                st[:, c * 512 : (c + 1) * 512],
                                kt_sb[hp][hl * 64 : (hl + 1) * 64, ub * 128 : (ub + 1) * 128],
                                qt_sb[hp][hl * 64 : (hl + 1) * 64, t0 + c * 512 : t0 + (c + 1) * 512],
                                start=True,
                                stop=True,
                            )
                        e = e_pool.tile(
                            [128, 1024], dt.bfloat16, tag="e", name=f"e_{hp}_{th}_{ub}_{hl}"
                        )
                        nc.scalar.activation(e[:], st[:], AF.Exp)
                        es.append(e)
                    for hl in range(2):
                        lh = 2 * hp + hl  # local head index 0..3
                        voff = ub * VROW + lh * VW
                        for c in range(2):
                            nc.tensor.matmul(
                                av[(hl, c)][:],
                                v_sb[:, voff : voff + VW],
                                es[hl][:, c * 512 : (c + 1) * 512],
                                start=(ub == 0),
                                stop=(ub == TB - 1),
                            )
                # evacuate av psum; pack the four D rows (hl,c) onto 32-aligned
                # partitions of one tile so a single multi-lane reciprocal
                # covers them all, off the PE critical path.
                avss = {}
                for hl in range(2):
                    avs = avs_pool.tile(
                        [64, 1024], dt.float32, tag="avs", name=f"avs_{hp}_{th}_{hl}"
                    )
                    for c in range(2):
                        csl = slice(c * 512, (c + 1) * 512)
                        if c == 0:
                            nc.vector.tensor_copy(avs[:, csl], av[(hl, c)][0:64, :])
                        else:
                            nc.scalar.copy(avs[:, csl], av[(hl, c)][0:64, :])
                    avss[hl] = avs
                # D rows parked at partitions {0,32,64,96}; unused partitions
                # preset to 1.0 so the full-tile reciprocal stays finite.
                drows = dinv_pool.tile(
                    [128, 512], dt.float32, tag="drows", name=f"drows_{hp}_{th}"
                )
                nc.gpsimd.memset(drows[:], 1.0)
                for hl in range(2):
                    for c in range(2):
                        r = hl * 64 + c * 32
                        if c == 0:
                            nc.vector.tensor_copy(
                                drows[r : r + 1, :], av[(hl, c)][64:65, :]
                            )
                        else:
                            nc.scalar.copy(drows[r : r + 1, :], av[(hl, c)][64:65, :])
                dinv = dinv_pool.tile(
                    [128, 512], dt.float32, tag="dinvf", name=f"dinvf_{hp}_{th}"
                )
                nc.vector.reciprocal(dinv[:], drows[:])
                dinvb = dinv_pool.tile(
                    [128, 512], dt.bfloat16, tag="dinvb", name=f"dinvb_{hp}_{th}"
                )
                nc.vector.tensor_copy(dinvb[:], dinv[:])
                return avss, dinvb

            def normalize_flush(hp, th, pend):
                t0 = th * 1024
                yt = yt_sb[hp]
                avss, dinvb = pend
                for c in range(2):
                    # broadcast 1/D of both heads over 128 partitions at once
                    dbp = ps1_pool.tile(
                        [128, 512], dt.float32, tag="ps1", name=f"dbp_{hp}_{th}_{c}"
                    )
                    nc.tensor.matmul(
                        dbp[:], sel[c][:], dinvb[:], start=True, stop=True
                    )
                    for hl in range(2):
                        nc.vector.tensor_tensor(
                            yt[hl * 64 : (hl + 1) * 64, t0 + c * 512 : t0 + (c + 1) * 512],
                            avss[hl][0:64, c * 512 : (c + 1) * 512],
                            dbp[hl * 64 : (hl + 1) * 64, :],
                            op=Alu.mult,
                        )

            def outproj(tb):
                osb = osb_pool.tile([128, K], dt.float32, tag="osb", name=f"osb_{tb}")
                pso = [
                    ps1_pool.tile([128, 512], dt.float32, tag="ps1", name=f"pso_{tb}_{i}")
                    for i in range(2)
                ]
                for hp in range(2):
                    for oc in range(2):
                        nc.tensor.matmul(
                            pso[oc][:],
                            yt_sb[hp][:, tb * 128 : (tb + 1) * 128],
                            wo_sb[hp][:, oc * 512 : (oc + 1) * 512],
                            start=(hp == 0),
                            stop=(hp == 1),
                        )
                for oc in range(2):
                    if tb % 2 == 0:
                        nc.vector.tensor_copy(osb[:, oc * 512 : (oc + 1) * 512], pso[oc][:])
                    else:
                        nc.scalar.copy(osb[:, oc * 512 : (oc + 1) * 512], pso[oc][:])
                nc.sync.dma_start(out[tb * 128 : (tb + 1) * 128, :], osb[:])

            qk_proj(0)
            pend00 = attention_uloop(0, 0)
            qk_proj(1)
            pend10 = attention_uloop(1, 0)
            normalize_flush(0, 0, pend00)
            normalize_flush(1, 0, pend10)
            for tb in range(0, 8):
                outproj(tb)
            pend01 = attention_uloop(0, 1)
            pend11 = attention_uloop(1, 1)
            normalize_flush(0, 1, pend01)
            normalize_flush(1, 1, pend11)
            for tb in range(8, 16):
                outproj(tb)

    nc.finalize()
    return nc


def _prepare_in_maps(x, Wq, Wk, Wv, Wo):
    scale = 1.0 / math.sqrt(K)
    xT = [np.ascontiguousarray(x[b].T).astype(_BF16) for b in range(B)]
    in_maps = []
    for c in range(NCORES):
        b = c // 4
        g = c % 4
        sl = slice(g * 256, (g + 1) * 256)
        in_maps.append(
            {
                "xT": xT[b],
                "wq": np.ascontiguousarray((Wq[sl, :].astype(np.float64) * scale).T).astype(_BF16),
                "wk": np.ascontiguousarray(Wk[sl, :].T).astype(_BF16),
                "wv": np.ascontiguousarray(Wv[sl, :].T).astype(_BF16),
                "wo": np.ascontiguousarray(Wo[:, sl].T).astype(_BF16),
            }
        )
    return in_maps


def _gather(results, bo):
    out = np.zeros((B, T, K), dtype=np.float32)
    for b in range(B):
        acc = np.zeros((T, K), dtype=np.float32)
        for g in range(4):
            acc += results[b * 4 + g]["out"].astype(np.float32)
        out[b] = acc + bo.astype(np.float32)[None, :]
    return out


def _maybe_enable_ldw_opt():
    import os
    import concourse.bass_utils as bu

    if os.environ.get("LDWOPT", "0") != "1":
        return
    if getattr(bu, "_ldwopt_patched", False):
        return
    orig = bu.run_command

    def patched(argv, **kw):
        argv = [
            "--enable-ldw-opt=true" if a == "--enable-ldw-opt=false" else a
            for a in argv
        ]
        return orig(argv, **kw)

    bu.run_command = patched
    bu._ldwopt_patched = True


def run(x, Wq, Wk, Wv, Wo, bo, trace=False, tmpdir=None):
    from concourse.bass_utils import run_bass_kernel_spmd

    _maybe_enable_ldw_opt()

    nc = build_program()
    in_maps = _prepare_in_maps(
        np.asarray(x), np.asarray(Wq), np.asarray(Wk), np.asarray(Wv), np.asarray(Wo)
    )
    res = run_bass_kernel_spmd(
        nc, in_maps, list(range(NCORES)), trace=trace, tmpdir=tmpdir
    )
    out = _gather(res.results, np.asarray(bo))
    return out, res


def kernel(x, Wq, Wk, Wv, Wo, bo):
    out, _ = run(x, Wq, Wk, Wv, Wo, bo, trace=False)
    return out
